# revision 14
# baseline (speedup 1.0000x reference)
"""Trainium2 Bass kernel for nn_DualGridMLMMelHarm.

Data-parallel over batch: 32 samples -> 8 cores x 4 samples. All params
replicated. Per core, a token-major fp32 residual stream [128, 8, 512]
(t = 4 samples x 256 tokens tiled by 128) flows through the dual encoder;
matmuls run in bf16 with fp32 PSUM accumulation.

Layout conventions (per core):
  stream  : [128(t), 8(tt), 512(d)]  fp32, token-major residual stream
  xT      : [128(d), 4(ko), 1024(t)] bf16, feature-major transposed acts
  qT/kT   : [128(o), 4(ot), 1024(t)] bf16, feature-major Q.T / K.T
  vTok    : [128(t), 8(tt), 512(o)]  bf16, token-major V
  A       : [128(q), 8(h), 256(k)]   bf16 per (sample, q-tile)
  aT      : [128(k), 8(h), 2(kt), 256(q)] bf16 per sample
Host pre-packs all weights transposed into the exact SBUF layouts.
"""

import os
import numpy as np
import ml_dtypes

# Model dims (hardcoded per the problem spec)
B, LM, LH = 32, 256, 256
D, NH, DFF, V, PR = 512, 8, 2048, 512, 13
NL_MEL, NL_HARM = 8, 8
P = 128
DH = D // NH          # 64
S = 4                 # samples per core
T = S * LM            # 1024 tokens per core
TT = T // P           # 8 token tiles
KO = D // P           # 4
FO = DFF // P         # 16
VO = V // P           # 4
NCH = 512             # matmul free-dim chunk over tokens
EPS = 1e-5
N_CORES = 8

_BUILD_CACHE = {}
ABLATE = set()          # debug: subset of {"attn", "ffn"}


def _sinpe(L, d):
    pos = np.arange(L, dtype=np.float32)[:, None]
    div = np.exp(np.arange(0, d, 2, dtype=np.float32) * (-np.log(10000.0) / d))
    ang = pos * div
    out = np.stack([np.sin(ang), np.cos(ang)], axis=-1).reshape(L, d)
    return out.astype(np.float32)


def _to_pko(w):
    """[K, O] -> [128, K//128, O] (partition-inner K)."""
    K, O = w.shape
    return np.ascontiguousarray(w.reshape(K // P, P, O).transpose(1, 0, 2))


def _col_pack(b):
    """[O] -> [128, O//128] per-partition scalar pack."""
    return np.ascontiguousarray(b.reshape(-1, P).T)


def _bf(x):
    return np.ascontiguousarray(x).astype(ml_dtypes.bfloat16)


def _build(flags):
    """Build the Bass module. flags = (fb_mel, fb_harm, fb_head, ln_affine)."""
    import concourse.bacc as bacc
    import concourse.mybir as mybir
    import concourse.tile as tile
    from concourse.masks import make_identity

    fb_mel, fb_harm, fb_head, ln_affine = flags
    F32 = mybir.dt.float32
    BF = mybir.dt.bfloat16
    AF = mybir.ActivationFunctionType
    ALU = mybir.AluOpType
    AX = mybir.AxisListType

    nc = bacc.Bacc("TRN2", target_bir_lowering=False)

    # ---------------- DRAM inputs (host-packed layouts) ----------------
    d_melT = nc.dram_tensor("melT_in", [PR, T], BF, kind="ExternalInput")
    d_wmel = nc.dram_tensor("wmelT", [PR, D], BF, kind="ExternalInput")
    d_oh = nc.dram_tensor("onehotT", [P, VO, T], BF, kind="ExternalInput")
    d_emb = nc.dram_tensor("emb", [P, VO, D], BF, kind="ExternalInput")
    d_pe = nc.dram_tensor("pe_tok", [P, 2, D], F32, kind="ExternalInput")

    d_mqkv = nc.dram_tensor("m_wqkv", [NL_MEL, P, KO, 3 * D], BF, kind="ExternalInput")
    d_mbqk = nc.dram_tensor("m_bqk", [NL_MEL, P, 8], F32, kind="ExternalInput")
    d_mwo = nc.dram_tensor("m_wo", [NL_MEL, P, KO, D], BF, kind="ExternalInput")
    d_mw1 = nc.dram_tensor("m_w1", [NL_MEL, P, KO, DFF], BF, kind="ExternalInput")
    d_mb1 = nc.dram_tensor("m_b1", [NL_MEL, P, FO], F32, kind="ExternalInput")
    d_mw2 = nc.dram_tensor("m_w2", [NL_MEL, P, FO, D], BF, kind="ExternalInput")

    d_hqkv = nc.dram_tensor("h_wqkv", [NL_HARM, P, KO, 3 * D], BF, kind="ExternalInput")
    d_hbqk = nc.dram_tensor("h_bqk", [NL_HARM, P, 8], F32, kind="ExternalInput")
    d_hwo = nc.dram_tensor("h_wo", [NL_HARM, P, KO, D], BF, kind="ExternalInput")
    d_hcq = nc.dram_tensor("h_wcq", [NL_HARM, P, KO, D], BF, kind="ExternalInput")
    d_hbcq = nc.dram_tensor("h_bcq", [NL_HARM, P, 4], F32, kind="ExternalInput")
    d_hckv = nc.dram_tensor("h_wckv", [NL_HARM, P, KO, 2 * D], BF, kind="ExternalInput")
    d_hbck = nc.dram_tensor("h_bck", [NL_HARM, P, 4], F32, kind="ExternalInput")
    d_hco = nc.dram_tensor("h_wco", [NL_HARM, P, KO, D], BF, kind="ExternalInput")
    d_hw1 = nc.dram_tensor("h_w1", [NL_HARM, P, KO, DFF], BF, kind="ExternalInput")
    d_hb1 = nc.dram_tensor("h_b1", [NL_HARM, P, FO], F32, kind="ExternalInput")
    d_hw2 = nc.dram_tensor("h_w2", [NL_HARM, P, FO, D], BF, kind="ExternalInput")

    d_whd = nc.dram_tensor("w_head", [P, KO, V], BF, kind="ExternalInput")

    # optional free-dim bias rows [1, 512] and LN affine [n, 2, 128, 512]
    if fb_mel:
        d_mbo = nc.dram_tensor("m_bo", [NL_MEL, 1, D], BF, kind="ExternalInput")
        d_mb2 = nc.dram_tensor("m_b2", [NL_MEL, 1, D], BF, kind="ExternalInput")
    if fb_harm:
        d_hbo = nc.dram_tensor("h_bo", [NL_HARM, 1, D], BF, kind="ExternalInput")
        d_hbco = nc.dram_tensor("h_bco", [NL_HARM, 1, D], BF, kind="ExternalInput")
        d_hb2 = nc.dram_tensor("h_b2", [NL_HARM, 1, D], BF, kind="ExternalInput")
    if fb_mel or fb_harm or fb_head:
        d_ones = nc.dram_tensor("ones_col", [1, P], BF, kind="ExternalInput")
    if fb_head:
        d_bhd = nc.dram_tensor("b_head", [1, V], BF, kind="ExternalInput")
    if ln_affine:
        n_ln = 4 + 2 * NL_MEL + 3 * NL_HARM
        d_ln = nc.dram_tensor("ln_gb", [n_ln, 2, P, D], F32, kind="ExternalInput")

    d_y = nc.dram_tensor("y", [T, V], F32, kind="ExternalOutput")

    from contextlib import ExitStack
    with tile.TileContext(nc) as tc, ExitStack() as ctx:
        consts = ctx.enter_context(tc.tile_pool(name="consts", bufs=1))
        sp = ctx.enter_context(tc.tile_pool(name="stream", bufs=2))
        wp = ctx.enter_context(tc.tile_pool(name="w", bufs=1))
        wp1 = ctx.enter_context(tc.tile_pool(name="wffn", bufs=1))
        ap_ = ctx.enter_context(tc.tile_pool(name="acts", bufs=1))
        smp = ctx.enter_context(tc.tile_pool(name="small", bufs=2))
        ps_big = ctx.enter_context(tc.tile_pool(name="psb", bufs=2, space="PSUM"))
        ps_sc = ctx.enter_context(tc.tile_pool(name="pssc", bufs=3, space="PSUM"))
        ps_tr = ctx.enter_context(tc.tile_pool(name="pstr", bufs=1, space="PSUM"))
        ps_av = ctx.enter_context(tc.tile_pool(name="psav", bufs=1, space="PSUM"))

        identf = consts.tile([P, P], F32)
        make_identity(nc, identf[:])
        identb = consts.tile([P, P], BF)
        make_identity(nc, identb[:])
        pe_sb = consts.tile([P, 2, D], F32)
        nc.sync.dma_start(pe_sb[:], d_pe[:])
        eps_sb = consts.tile([P, 1], F32)
        nc.any.memset(eps_sb[:], EPS)
        if fb_mel or fb_harm or fb_head:
            ones_sb = consts.tile([1, P], BF)
            nc.sync.dma_start(ones_sb[:], d_ones[:])
        if ln_affine:
            ln_idx = [0]
            lng = consts.tile([P, D], F32, tag="lng")
            lnb = consts.tile([P, D], F32, tag="lnb")

        # ---------------- helpers ----------------
        def layernorm(stream):
            """returns new stream tile = LN(stream) (token-major)."""
            new = sp.tile([P, TT, D], F32, tag="stream")
            ssum = smp.tile([P, TT], F32, tag="ln_sum")
            ssq = smp.tile([P, TT], F32, tag="ln_ssq")
            nc.vector.tensor_reduce(ssum[:], stream[:], AX.X, ALU.add)
            sq = smp.tile([P, D], F32, tag="ln_sq")
            for tt in range(TT):
                nc.scalar.activation(sq[:], stream[:, tt, :], AF.Square,
                                     accum_out=ssq[:, tt:tt + 1])
            mean = smp.tile([P, TT], F32, tag="ln_mean")
            nc.vector.tensor_scalar_mul(mean[:], ssum[:], 1.0 / D)
            m2 = smp.tile([P, TT], F32, tag="ln_m2")
            nc.vector.tensor_tensor(m2[:], mean[:], mean[:], ALU.mult)
            var = smp.tile([P, TT], F32, tag="ln_var")
            nc.vector.tensor_scalar_mul(var[:], ssq[:], 1.0 / D)
            nc.vector.tensor_tensor(var[:], var[:], m2[:], ALU.subtract)
            std = smp.tile([P, TT], F32, tag="ln_std")
            nc.scalar.activation(std[:], var[:], AF.Sqrt, bias=eps_sb[:])
            rstd = smp.tile([P, TT], F32, tag="ln_rstd")
            nc.vector.reciprocal(rstd[:], std[:])
            negm = smp.tile([P, TT], F32, tag="ln_negm")
            nc.vector.tensor_tensor(negm[:], mean[:], rstd[:], ALU.mult)
            nc.vector.tensor_scalar_mul(negm[:], negm[:], -1.0)
            for tt in range(TT):
                nc.scalar.activation(new[:, tt, :], stream[:, tt, :], AF.Identity,
                                     bias=negm[:, tt:tt + 1],
                                     scale=rstd[:, tt:tt + 1])
            if ln_affine:
                i = ln_idx[0]
                ln_idx[0] += 1
                nc.sync.dma_start(lng[:], d_ln[i, 0])
                nc.sync.dma_start(lnb[:], d_ln[i, 1])
                for tt in range(TT):
                    nc.vector.tensor_tensor(new[:, tt, :], new[:, tt, :],
                                            lng[:], ALU.mult)
                    nc.vector.tensor_tensor(new[:, tt, :], new[:, tt, :],
                                            lnb[:], ALU.add)
            return new

        def to_xT(stream, tag="xT"):
            """fp32 token-major stream -> bf16 feature-major [128, KO, T]."""
            xT = ap_.tile([P, KO, T], BF, tag=tag)
            for tt in range(TT):
                for dc in range(KO):
                    pst = ps_tr.tile([P, P], F32, tag="tr")
                    nc.tensor.transpose(
                        pst[:], stream[:, tt, dc * P:(dc + 1) * P], identf[:])
                    nc.scalar.copy(xT[:, dc, tt * P:(tt + 1) * P], pst[:])
            return xT

        def lin_feat(out_sb, out_ot0, w_sb, wcol0, n_ot, bias=None):
            """feature-major linear: out.T[o, t] = W.T columns -> psum [o128, t512].
            out_sb [128, *, T] bf16; bias: (tile, col0) per-partition pack."""
            for i in range(n_ot):
                for ch in range(T // NCH):
                    pl = ps_big.tile([P, NCH], F32, tag="big")
                    for ko in range(KO):
                        nc.tensor.matmul(
                            pl[:],
                            w_sb[:, ko, wcol0 + i * P: wcol0 + (i + 1) * P],
                            xT_cur[0][:, ko, ch * NCH:(ch + 1) * NCH],
                            start=(ko == 0), stop=(ko == KO - 1))
                    if bias is not None:
                        bt, bc = bias
                        nc.scalar.activation(
                            out_sb[:, out_ot0 + i, ch * NCH:(ch + 1) * NCH],
                            pl[:], AF.Identity, bias=bt[:, bc + i:bc + i + 1])
                    else:
                        nc.scalar.copy(
                            out_sb[:, out_ot0 + i, ch * NCH:(ch + 1) * NCH], pl[:])

        def lin_tok_v(out_sb, w_sb, wcol0):
            """token-major V-style linear: psum [t128, 512]."""
            for tt in range(TT):
                pl = ps_big.tile([P, NCH], F32, tag="big")
                for ko in range(KO):
                    nc.tensor.matmul(
                        pl[:],
                        xT_cur[0][:, ko, tt * P:(tt + 1) * P],
                        w_sb[:, ko, wcol0:wcol0 + D],
                        start=(ko == 0), stop=(ko == KO - 1))
                nc.scalar.copy(out_sb[:, tt, :], pl[:])

        def attention(qT, kT, vTok, wo_sb, stream, bias_row):
            """full attention block; returns new stream = stream + attn_out."""
            attnT = ap_.tile([P, KO, T], BF, tag="attnT")
            for s in range(S):
                if "attn_all" in ABLATE:
                    break
                A = ap_.tile([P, NH, LM], BF, tag="A")
                sums = smp.tile([P, NH], F32, tag="sums")
                recip = smp.tile([P, NH], F32, tag="recip")
                for qt in range(2):
                    for h in range(NH if "attn_sm" not in ABLATE else 0):
                        base = (h % 2) * DH
                        sc = ps_sc.tile([P, LM], F32, tag="sc")
                        nc.tensor.matmul(
                            sc[:],
                            qT[base:base + DH, h // 2,
                               s * LM + qt * P: s * LM + (qt + 1) * P],
                            kT[base:base + DH, h // 2, s * LM:(s + 1) * LM],
                            start=True, stop=True)
                        if "attn_noexp" not in ABLATE:
                            nc.scalar.activation(
                                A[:, h, :], sc[:], AF.Exp,
                                accum_out=sums[:, h:h + 1])
                    if "attn_sm" in ABLATE or "attn_tr" in ABLATE:
                        continue
                    nc.vector.reciprocal(recip[:], sums[:])
                    nc.vector.tensor_tensor(
                        A[:], A[:], recip[:, :, None].to_broadcast((P, NH, LM)),
                        ALU.mult)
                    # transpose A -> aT for this q-tile
                    if qt == 0:
                        aT = ap_.tile([P, NH, 2, LM], BF, tag="aT")
                    for h in range(NH):
                        for kt in range(2):
                            pst = ps_tr.tile([P, P], BF, tag="trb")
                            nc.tensor.transpose(
                                pst[:], A[:, h, kt * P:(kt + 1) * P], identb[:])
                            nc.vector.tensor_copy(
                                aT[:, h, kt, qt * P:(qt + 1) * P], pst[:])
                # AV per head-pair
                if ABLATE & {"attn_sm", "attn_tr", "attn_av"}:
                    continue
                for hp in range(4):
                    av = ps_av.tile([P, LM], F32, tag="av")
                    for h2 in range(2):
                        h = hp * 2 + h2
                        for kt in range(2):
                            nc.tensor.matmul(
                                av[h2 * DH:(h2 + 1) * DH, :],
                                vTok[:, s * 2 + kt, h * DH:(h + 1) * DH],
                                aT[:, h, kt, :],
                                start=(kt == 0), stop=(kt == 1))
                    nc.scalar.copy(attnT[:, hp, s * LM:(s + 1) * LM], av[:])
            # out-proj (token-major) + residual
            if ABLATE & {"attn_all", "attn_sm", "attn_tr", "attn_av", "attn_out"}:
                return stream
            new = sp.tile([P, TT, D], F32, tag="stream")
            for tt in range(TT):
                pl = ps_big.tile([P, NCH], F32, tag="big")
                for ko in range(KO):
                    nc.tensor.matmul(
                        pl[:], attnT[:, ko, tt * P:(tt + 1) * P],
                        wo_sb[:, ko, :], start=(ko == 0),
                        stop=(ko == KO - 1 and bias_row is None))
                if bias_row is not None:
                    nc.tensor.matmul(pl[:], ones_sb[:], bias_row[:],
                                     start=False, stop=True)
                nc.vector.tensor_tensor(new[:, tt, :], pl[:], stream[:, tt, :],
                                        ALU.add)
            return new

        def ffn(w1_sb, b1_sb, w2_sb, stream, xT3, bias_row):
            new = sp.tile([P, TT, D], F32, tag="stream")
            for ch in range(T // NCH):
                hT = ap_.tile([P, FO, NCH], BF, tag="hT")
                for fo in range(FO):
                    pl = ps_big.tile([P, NCH], F32, tag="big")
                    for ko in range(KO):
                        nc.tensor.matmul(
                            pl[:], w1_sb[:, ko, fo * P:(fo + 1) * P],
                            xT3[:, ko, ch * NCH:(ch + 1) * NCH],
                            start=(ko == 0), stop=(ko == KO - 1))
                    nc.scalar.activation(hT[:, fo, :], pl[:], AF.Gelu,
                                         bias=b1_sb[:, fo:fo + 1])
                for tj in range(NCH // P):
                    tt = ch * (NCH // P) + tj
                    pl = ps_big.tile([P, NCH], F32, tag="big")
                    for fo in range(FO):
                        nc.tensor.matmul(
                            pl[:], hT[:, fo, tj * P:(tj + 1) * P], w2_sb[:, fo, :],
                            start=(fo == 0),
                            stop=(fo == FO - 1 and bias_row is None))
                    if bias_row is not None:
                        nc.tensor.matmul(pl[:], ones_sb[:], bias_row[:],
                                         start=False, stop=True)
                    nc.vector.tensor_tensor(new[:, tt, :], pl[:],
                                            stream[:, tt, :], ALU.add)
            return new

        def load_bias_row(dram, l):
            t = smp.tile([1, D], BF, tag="brow")
            nc.sync.dma_start(t[:], dram[l])
            return t

        xT_cur = [None]  # closure cell for lin_feat/lin_tok_v

        # ================= melody encoder =================
        melT_in = consts.tile([PR, T], BF)
        nc.sync.dma_start(melT_in[:], d_melT[:])
        wmel = consts.tile([PR, D], BF)
        nc.sync.dma_start(wmel[:], d_wmel[:])
        stream = sp.tile([P, TT, D], F32, tag="stream")
        for tt in range(TT):
            pl = ps_big.tile([P, NCH], F32, tag="big")
            nc.tensor.matmul(pl[:], melT_in[:, tt * P:(tt + 1) * P], wmel[:],
                             start=True, stop=True)
            nc.vector.tensor_tensor(stream[:, tt, :], pl[:],
                                    pe_sb[:, tt % 2, :], ALU.add)
        stream = layernorm(stream)

        for l in range(NL_MEL):
            wqkv = wp.tile([P, KO, 3 * D], BF, tag="wqkv")
            nc.sync.dma_start(wqkv[:], d_mqkv[l])
            bqk = smp.tile([P, 8], F32, tag="bqk")
            nc.sync.dma_start(bqk[:], d_mbqk[l])
            wo = wp.tile([P, KO, D], BF, tag="wo")
            nc.sync.dma_start(wo[:], d_mwo[l])

            xT = to_xT(stream)
            xT_cur[0] = xT
            qT = ap_.tile([P, KO, T], BF, tag="qT")
            kT = ap_.tile([P, KO, T], BF, tag="kT")
            vTok = ap_.tile([P, TT, D], BF, tag="vTok")
            lin_feat(qT, 0, wqkv, 0, KO, bias=(bqk, 0))
            lin_feat(kT, 0, wqkv, D, KO, bias=(bqk, 4))
            lin_tok_v(vTok, wqkv, 2 * D)
            bo = load_bias_row(d_mbo, l) if fb_mel else None
            if "attn" not in ABLATE:
                stream = attention(qT, kT, vTok, wo, stream, bo)
            stream = layernorm(stream)

            w1 = wp1.tile([P, KO, DFF], BF, tag="w1")
            nc.sync.dma_start(w1[:], d_mw1[l])
            b1 = smp.tile([P, FO], F32, tag="b1")
            nc.sync.dma_start(b1[:], d_mb1[l])
            w2 = wp1.tile([P, FO, D], BF, tag="w2")
            nc.sync.dma_start(w2[:], d_mw2[l])
            xT3 = to_xT(stream, tag="xT")
            b2 = load_bias_row(d_mb2, l) if fb_mel else None
            if "ffn" not in ABLATE:
                stream = ffn(w1, b1, w2, stream, xT3, b2)
            stream = layernorm(stream)

        stream = layernorm(stream)  # out_mel
        melT = consts.tile([P, KO, T], BF)  # resident mel context (feature-major)
        for tt in range(TT):
            for dc in range(KO):
                pst = ps_tr.tile([P, P], F32, tag="tr")
                nc.tensor.transpose(pst[:], stream[:, tt, dc * P:(dc + 1) * P],
                                    identf[:])
                nc.scalar.copy(melT[:, dc, tt * P:(tt + 1) * P], pst[:])

        # ================= harmony encoder =================
        oh = consts.tile([P, VO, T], BF)
        nc.sync.dma_start(oh[:], d_oh[:])
        emb = consts.tile([P, VO, D], BF)
        nc.sync.dma_start(emb[:], d_emb[:])
        stream = sp.tile([P, TT, D], F32, tag="stream")
        for tt in range(TT):
            pl = ps_big.tile([P, NCH], F32, tag="big")
            for vo in range(VO):
                nc.tensor.matmul(pl[:], oh[:, vo, tt * P:(tt + 1) * P],
                                 emb[:, vo, :], start=(vo == 0),
                                 stop=(vo == VO - 1))
            nc.vector.tensor_tensor(stream[:, tt, :], pl[:],
                                    pe_sb[:, tt % 2, :], ALU.add)
        stream = layernorm(stream)  # in_harm

        for l in range(NL_HARM):
            # ---- self attention (pre-norm) ----
            wqkv = wp.tile([P, KO, 3 * D], BF, tag="wqkv")
            nc.sync.dma_start(wqkv[:], d_hqkv[l])
            bqk = smp.tile([P, 8], F32, tag="bqk")
            nc.sync.dma_start(bqk[:], d_hbqk[l])
            wo = wp.tile([P, KO, D], BF, tag="wo")
            nc.sync.dma_start(wo[:], d_hwo[l])

            stream = layernorm(stream)  # ln1
            xT = to_xT(stream)
            xT_cur[0] = xT
            qT = ap_.tile([P, KO, T], BF, tag="qT")
            kT = ap_.tile([P, KO, T], BF, tag="kT")
            vTok = ap_.tile([P, TT, D], BF, tag="vTok")
            lin_feat(qT, 0, wqkv, 0, KO, bias=(bqk, 0))
            lin_feat(kT, 0, wqkv, D, KO, bias=(bqk, 4))
            lin_tok_v(vTok, wqkv, 2 * D)
            bo = load_bias_row(d_hbo, l) if fb_harm else None
            if "attn" not in ABLATE:
                stream = attention(qT, kT, vTok, wo, stream, bo)

            # ---- cross attention ----
            wcq = wp.tile([P, KO, D], BF, tag="wcq")
            nc.sync.dma_start(wcq[:], d_hcq[l])
            bcq = smp.tile([P, 4], F32, tag="bcq")
            nc.sync.dma_start(bcq[:], d_hbcq[l])
            wckv = wp.tile([P, KO, 2 * D], BF, tag="wckv")
            nc.sync.dma_start(wckv[:], d_hckv[l])
            bck = smp.tile([P, 4], F32, tag="bck")
            nc.sync.dma_start(bck[:], d_hbck[l])
            wco = wp.tile([P, KO, D], BF, tag="wco")
            nc.sync.dma_start(wco[:], d_hco[l])

            stream = layernorm(stream)  # ln2
            xT = to_xT(stream)
            xT_cur[0] = xT
            cqT = ap_.tile([P, KO, T], BF, tag="qT")
            lin_feat(cqT, 0, wcq, 0, KO, bias=(bcq, 0))
            # cross K (feature-major from melT) and V (token-major from melT)
            ckT = ap_.tile([P, KO, T], BF, tag="kT")
            cvT = ap_.tile([P, TT, D], BF, tag="vTok")
            xT_cur[0] = melT
            lin_feat(ckT, 0, wckv, 0, KO, bias=(bck, 0))
            lin_tok_v(cvT, wckv, D)
            bco = load_bias_row(d_hbco, l) if fb_harm else None
            if "attn" not in ABLATE:
                stream = attention(cqT, ckT, cvT, wco, stream, bco)

            # ---- FFN ----
            w1 = wp1.tile([P, KO, DFF], BF, tag="w1")
            nc.sync.dma_start(w1[:], d_hw1[l])
            b1 = smp.tile([P, FO], F32, tag="b1")
            nc.sync.dma_start(b1[:], d_hb1[l])
            w2 = wp1.tile([P, FO, D], BF, tag="w2")
            nc.sync.dma_start(w2[:], d_hw2[l])
            stream = layernorm(stream)  # ln3
            xT3 = to_xT(stream)
            b2 = load_bias_row(d_hb2, l) if fb_harm else None
            if "ffn" not in ABLATE:
                stream = ffn(w1, b1, w2, stream, xT3, b2)

        stream = layernorm(stream)  # out_harm
        # ---- head ----
        whd = consts.tile([P, KO, V], BF)
        nc.sync.dma_start(whd[:], d_whd[:])
        bhd = None
        if fb_head:
            bhd = smp.tile([1, V], BF, tag="brow")
            nc.sync.dma_start(bhd[:], d_bhd[:])
        xTh = to_xT(stream)
        for tt in range(TT):
            pl = ps_big.tile([P, V], F32, tag="big")
            for ko in range(KO):
                nc.tensor.matmul(pl[:], xTh[:, ko, tt * P:(tt + 1) * P],
                                 whd[:, ko, :], start=(ko == 0),
                                 stop=(ko == KO - 1 and bhd is None))
            if bhd is not None:
                nc.tensor.matmul(pl[:], ones_sb[:], bhd[:], start=False,
                                 stop=True)
            out_sb = smp.tile([P, V], F32, tag="out")
            nc.vector.tensor_copy(out_sb[:], pl[:])
            nc.sync.dma_start(d_y[tt * P:(tt + 1) * P, :], out_sb[:])

    nc.compile()
    return nc


def _get_nc(flags):
    if flags not in _BUILD_CACHE:
        _BUILD_CACHE[flags] = _build(flags)
    return _BUILD_CACHE[flags]


def _prepare(inputs):
    """Host-side packing: returns (in_maps per core, flags)."""
    mel_grid = np.asarray(inputs["melody_grid"], np.float32)
    toks = np.asarray(inputs["harmony_tokens"])
    mel_proj = {k: np.asarray(v, np.float32) for k, v in inputs["mel_proj"].items()}
    harm_emb = np.asarray(inputs["harm_emb"], np.float32)
    mls = {k: np.asarray(v, np.float32) for k, v in inputs["mel_layers"].items()}
    hls = {k: np.asarray(v, np.float32) for k, v in inputs["harm_layers"].items()}
    norms = {k: np.asarray(v, np.float32) for k, v in inputs["norms"].items()}
    head = {k: np.asarray(v, np.float32) for k, v in inputs["out_head"].items()}

    sc = 1.0 / np.sqrt(DH)
    pe = _sinpe(max(LM, LH), D)

    def nz(*arrs):
        return any(np.abs(a).max() > 0 for a in arrs)

    # mel per-layer packs
    m_wqkv, m_bqk, m_wo, m_w1, m_b1, m_w2 = [], [], [], [], [], []
    m_bo, m_b2 = [], []
    for i in range(NL_MEL):
        qkv_w = mls["qkv_w"][i].copy()   # [3D, D]
        qkv_b = mls["qkv_b"][i].copy()
        qkv_w[:D] *= sc
        qkv_b[:D] *= sc
        wq, wk, wv = qkv_w[:D], qkv_w[D:2 * D], qkv_w[2 * D:]
        bv = qkv_b[2 * D:]
        wqkvT = np.concatenate([wq.T, wk.T, wv.T], axis=1)  # [D, 3D]
        m_wqkv.append(_to_pko(wqkvT))
        m_bqk.append(np.concatenate(
            [_col_pack(qkv_b[:D]), _col_pack(qkv_b[D:2 * D])], axis=1))
        m_wo.append(_to_pko(mls["out_w"][i].T))
        m_bo.append((mls["out_b"][i] + mls["out_w"][i] @ bv).reshape(1, D))
        m_w1.append(_to_pko(mls["l1_w"][i].T))
        m_b1.append(_col_pack(mls["l1_b"][i]))
        m_w2.append(_to_pko(mls["l2_w"][i].T))
        m_b2.append(mls["l2_b"][i].reshape(1, D))

    h_wqkv, h_bqk, h_wo, h_w1, h_b1, h_w2 = [], [], [], [], [], []
    h_cq, h_bcq, h_ckv, h_bck, h_co = [], [], [], [], []
    h_bo, h_bco, h_b2 = [], [], []
    for i in range(NL_HARM):
        qkv_w = hls["qkv_w"][i].copy()
        qkv_b = hls["qkv_b"][i].copy()
        qkv_w[:D] *= sc
        qkv_b[:D] *= sc
        wq, wk, wv = qkv_w[:D], qkv_w[D:2 * D], qkv_w[2 * D:]
        bv = qkv_b[2 * D:]
        h_wqkv.append(_to_pko(np.concatenate([wq.T, wk.T, wv.T], axis=1)))
        h_bqk.append(np.concatenate(
            [_col_pack(qkv_b[:D]), _col_pack(qkv_b[D:2 * D])], axis=1))
        h_wo.append(_to_pko(hls["out_w"][i].T))
        h_bo.append((hls["out_b"][i] + hls["out_w"][i] @ bv).reshape(1, D))
        cqkv_w = hls["cqkv_w"][i].copy()
        cqkv_b = hls["cqkv_b"][i].copy()
        cqkv_w[:D] *= sc
        cqkv_b[:D] *= sc
        cbv = cqkv_b[2 * D:]
        h_cq.append(_to_pko(cqkv_w[:D].T))
        h_bcq.append(_col_pack(cqkv_b[:D]))
        h_ckv.append(_to_pko(np.concatenate(
            [cqkv_w[D:2 * D].T, cqkv_w[2 * D:].T], axis=1)))
        h_bck.append(_col_pack(cqkv_b[D:2 * D]))
        h_co.append(_to_pko(hls["cout_w"][i].T))
        h_bco.append((hls["cout_b"][i] + hls["cout_w"][i] @ cbv).reshape(1, D))
        h_w1.append(_to_pko(hls["l1_w"][i].T))
        h_b1.append(_col_pack(hls["l1_b"][i]))
        h_w2.append(_to_pko(hls["l2_w"][i].T))
        h_b2.append(hls["l2_b"][i].reshape(1, D))

    fb_mel = nz(*m_bo, *m_b2)
    fb_harm = nz(*h_bo, *h_bco, *h_b2)
    fb_head = nz(head["b"])

    ln_list = [
        (norms["in_mel_g"], norms["in_mel_b"]),
    ]
    for i in range(NL_MEL):
        ln_list += [(mls["ln1_g"][i], mls["ln1_b"][i]),
                    (mls["ln2_g"][i], mls["ln2_b"][i])]
    ln_list += [(norms["out_mel_g"], norms["out_mel_b"]),
                (norms["in_harm_g"], norms["in_harm_b"])]
    for i in range(NL_HARM):
        ln_list += [(hls["ln1_g"][i], hls["ln1_b"][i]),
                    (hls["ln2_g"][i], hls["ln2_b"][i]),
                    (hls["ln3_g"][i], hls["ln3_b"][i])]
    ln_list += [(norms["out_harm_g"], norms["out_harm_b"])]
    ln_affine = any(np.abs(g - 1.0).max() > 0 or np.abs(b).max() > 0
                    for g, b in ln_list)

    flags = (bool(fb_mel), bool(fb_harm), bool(fb_head), bool(ln_affine))

    # shared (replicated) tensors
    shared = {
        "wmelT": _bf(mel_proj["w"].T),                       # [13, 512]
        "emb": _bf(_to_pko(harm_emb)),                       # [128, 4, 512]
        "pe_tok": np.ascontiguousarray(
            pe[:LM].reshape(2, P, D).transpose(1, 0, 2)),    # [128, 2, 512]
        "m_wqkv": _bf(np.stack(m_wqkv)), "m_bqk": np.stack(m_bqk),
        "m_wo": _bf(np.stack(m_wo)),
        "m_w1": _bf(np.stack(m_w1)), "m_b1": np.stack(m_b1),
        "m_w2": _bf(np.stack(m_w2)),
        "h_wqkv": _bf(np.stack(h_wqkv)), "h_bqk": np.stack(h_bqk),
        "h_wo": _bf(np.stack(h_wo)),
        "h_wcq": _bf(np.stack(h_cq)), "h_bcq": np.stack(h_bcq),
        "h_wckv": _bf(np.stack(h_ckv)), "h_bck": np.stack(h_bck),
        "h_wco": _bf(np.stack(h_co)),
        "h_w1": _bf(np.stack(h_w1)), "h_b1": np.stack(h_b1),
        "h_w2": _bf(np.stack(h_w2)),
        "w_head": _bf(_to_pko(head["w"].T)),                 # [128, 4, 512]
    }
    # mel-proj bias folded into pe (it is zero in practice but fold anyway)
    shared["pe_tok"] = shared["pe_tok"] + mel_proj["b"].astype(np.float32)
    if flags[0]:
        shared["m_bo"] = _bf(np.stack(m_bo))
        shared["m_b2"] = _bf(np.stack(m_b2))
    if flags[1]:
        shared["h_bo"] = _bf(np.stack(h_bo))
        shared["h_bco"] = _bf(np.stack(h_bco))
        shared["h_b2"] = _bf(np.stack(h_b2))
    if flags[2]:
        shared["b_head"] = _bf(head["b"].reshape(1, V))
    if flags[0] or flags[1] or flags[2]:
        shared["ones_col"] = _bf(np.ones((1, P), np.float32))
    if flags[3]:
        gb = np.stack([np.stack([np.broadcast_to(g, (P, D)),
                                 np.broadcast_to(b, (P, D))])
                       for g, b in ln_list])
        shared["ln_gb"] = np.ascontiguousarray(gb.astype(np.float32))

    in_maps = []
    for c in range(N_CORES):
        s0 = c * S
        mg = mel_grid[s0:s0 + S]                        # [4, 256, 13]
        melT = mg.reshape(T, PR).T                      # [13, 1024]
        tk = np.asarray(toks[s0:s0 + S]).reshape(T)     # [1024]
        ohT = (tk[None, :] == np.arange(V)[:, None])    # [512, 1024]
        oh_p = ohT.reshape(VO, P, T).transpose(1, 0, 2) # [128, 4, 1024]
        m = dict(shared)
        m["melT_in"] = _bf(melT)
        m["onehotT"] = _bf(oh_p.astype(np.float32))
        in_maps.append(m)
    return in_maps, flags


def kernel(**inputs):
    from concourse.bass_utils import run_bass_kernel_spmd
    in_maps, flags = _prepare(inputs)
    nc = _get_nc(flags)
    res = run_bass_kernel_spmd(nc, in_maps, core_ids=list(range(N_CORES)))
    out = np.stack([np.asarray(r["y"], np.float32).reshape(S, LH, V)
                    for r in res.results])
    return np.ascontiguousarray(out.reshape(B, LH, V))
